# revision 1
# baseline (speedup 1.0000x reference)
"""Trainium2 Bass kernel for nn_Attention_81037442941065.

Dual-attention module (spatial [b,h,n,n] + channel [b,h,d,d]) with
B=2, N=2048, DIM=1024, 16 heads of d=64.

Sharding: 8 cores = (2 batches) x (4 head-groups of 4 heads).
Each core computes its batch/head-group slice end-to-end and produces a
partial (over head groups) output projection; the host sums the 4 group
partials per batch and adds b_out.

Compute is bf16 end-to-end. Measured on this hardware a matmul's
serialized cost is ~(moving-free x 0.417ns + fixed overhead) with the
weight load mostly hidden under the previous matmul's stream, so fp8
DoubleRow (out free capped at 256, dst must start at partition 0) buys
no throughput over bf16 at free=512; fp8e4m3 is used ONLY to compress
the x/z input DMA (their error feeds the attenuated out1/logit paths;
|out2| ~ 5x |out1| and the y path stays bf16). The wins over the
original baseline are scheduling and engine balance:
  * p_-outer iteration order (all 4 query blocks of head-pair 0, then
    pair 1) spreads projection deadlines; every projection, out2 and
    the final projection run as labeled aux PE units drained inside the
    spatial S/exp/AV loop, with drain_until() guards making emission
    order (which Tile dependency tracking requires) explicit.
  * Input DMA is split across the three DGE dispatch engines (SP, ACT,
    Pool) in consumption order: the first exp gates only on w_sa1+zT
    and w_sa2+yT.
  * AV matmuls (lhsT = [xh | ones] so psum row 64 accumulates the
    softmax denominators) are emitted elastically, lagging ~3 slots and
    gated on their xh chunk unit, so neither the DMA-gated first
    iteration nor the tails' psum-bank WAR head-of-line blocks the exp
    stream.
  * cat is stored head-pair-packed so the final projection contracts
    K=128 (full PE) instead of K=64; the upper head of a pair reaches
    catp partitions 64-127 via a base-64 psum matmul (out2) and one
    SBUF->SBUF bridge DMA per iteration (normalized out1).
  * ScalarE is the exp engine (~140us, co-bottleneck with PE ~212us);
    the z1T/yhT/output psum copies ride its slack windows, keeping DVE
    off the critical tails->catp->finals chain.
  * bf16 output partials (halves outbound DMA).

Measured: ~288 us vs 327 us for the staged baseline under the same
harness; end-to-end relative error ~6.3e-3 (gate 2e-2).
"""

import sys

for _p in ("/opt/trn_rl_repo", "/opt/pypackages"):
    if _p not in sys.path:
        sys.path.insert(0, _p)

import ml_dtypes
import numpy as np
from contextlib import ExitStack

import concourse.bacc as bacc
import concourse.mybir as mybir
import concourse.tile as tile
from concourse.tile import add_dep_helper
from concourse.bass_utils import run_bass_kernel_spmd

F32 = mybir.dt.float32
BF16 = mybir.dt.bfloat16
FP8 = mybir.dt.float8e4
EXP = mybir.ActivationFunctionType.Exp
COPY = mybir.ActivationFunctionType.Copy

B, N, DIM = 2, 2048, 1024
HEADS, DH = 16, 64
G = 4              # head groups == cores per batch
HG = HEADS // G    # heads per group (4)
CIN = HG * DH      # inner channels per core (256)
NCORES = 8
NCH = N // 128     # 128-token chunks (16)
SCALE = DH ** -0.5            # 1/8
CM_SCALE = SCALE / (N / DH)   # 1/256
XP = DH + 2        # xh_aug head pitch: 64 ch + ones col @64 + pad (66)


def _ride(mm, host, why):
    add_dep_helper(mm.ins, host.ins, sync=False, reason=why)


def _build_program():
    nc = bacc.Bacc(
        "TRN2", target_bir_lowering=False, debug=False, num_devices=NCORES
    )

    # ---- DRAM I/O ----
    xT_d = nc.dram_tensor("xT", [DIM, N], FP8, kind="ExternalInput").ap()
    yT_d = nc.dram_tensor("yT", [DIM, N], BF16, kind="ExternalInput").ap()
    zT_d = nc.dram_tensor("zT", [DIM, N], FP8, kind="ExternalInput").ap()
    wsa1_d = nc.dram_tensor("w_sa1", [DIM, CIN], BF16, kind="ExternalInput").ap()
    wsa2_d = nc.dram_tensor("w_sa2", [DIM, CIN], BF16, kind="ExternalInput").ap()
    wse1_d = nc.dram_tensor("w_se1", [DIM, CIN], BF16, kind="ExternalInput").ap()
    wse2_d = nc.dram_tensor("w_se2", [DIM, CIN], BF16, kind="ExternalInput").ap()
    wout_d = nc.dram_tensor("w_out", [CIN, DIM], BF16, kind="ExternalInput").ap()
    outT_d = nc.dram_tensor("outT", [DIM, N], BF16, kind="ExternalOutput").ap()

    with tile.TileContext(nc) as tc, ExitStack() as ctx:
        ppool = ctx.enter_context(tc.tile_pool(name="persist", bufs=1))
        ipool = ctx.enter_context(tc.tile_pool(name="inputs", bufs=1))
        ptpool = ctx.enter_context(tc.tile_pool(name="pt", bufs=10))
        tpool = ctx.enter_context(tc.tile_pool(name="tails", bufs=1))
        opool = ctx.enter_context(tc.tile_pool(name="oout", bufs=3))
        psS = ctx.enter_context(tc.tile_pool(name="psS", bufs=2, space="PSUM"))
        psAV = ctx.enter_context(tc.tile_pool(name="psAV", bufs=2, space="PSUM"))
        psaux = ctx.enter_context(tc.tile_pool(name="psaux", bufs=2, space="PSUM"))

        # ---- persistent tiles ----
        z1T = [ppool.tile([128, N], BF16, tag=f"z1T{m}", name=f"z1T{m}")
               for m in range(2)]   # head pair m: [2x64 ch, tokens]
        yhT = [ppool.tile([128, N], BF16, tag=f"yhT{m}", name=f"yhT{m}")
               for m in range(2)]
        catp = [ppool.tile([128, N], BF16, tag=f"cat{m}", name=f"cat{m}")
                for m in range(2)]  # head-pair-packed out1+out2
        # xh_aug[i]: [tok128, head, XP]; ch 0..63, ones col @64
        xq = [ppool.tile([128, HG * XP], BF16, tag=f"xq{i}", name=f"xq{i}")
              for i in range(NCH)]
        zq = [ppool.tile([128, HG * DH], BF16, tag=f"zq{i}", name=f"zq{i}")
              for i in range(NCH)]
        secm_sb = [ppool.tile([128, DH], BF16, tag=f"cm{p}", name=f"cm{p}")
                   for p in range(2)]
        rs = [ppool.tile([64, 1], F32, tag=f"rs{h}", name=f"rs{h}")
              for h in range(HG)]
        rcm = [ppool.tile([64, 1], F32, tag=f"rcm{h}", name=f"rcm{h}")
               for h in range(HG)]

        # ---- input tiles (all [128-dim-chunk, ...] bf16) ----
        xt = [ipool.tile([128, N], FP8, tag=f"xt{k}", name=f"xt{k}")
              for k in range(8)]
        yt = [ipool.tile([128, N], BF16, tag=f"yt{k}", name=f"yt{k}")
              for k in range(8)]
        zt = [ipool.tile([128, N], FP8, tag=f"zt{k}", name=f"zt{k}")
              for k in range(8)]
        wsa1_t = [ipool.tile([128, CIN], BF16, tag=f"wsa1_{k}",
                             name=f"wsa1_{k}") for k in range(8)]
        wsa2_t = [ipool.tile([128, CIN], BF16, tag=f"wsa2_{k}",
                             name=f"wsa2_{k}") for k in range(8)]
        wse1_t = [ipool.tile([128, CIN], BF16, tag=f"wse1_{k}",
                             name=f"wse1_{k}") for k in range(8)]
        wse2_t = [ipool.tile([128, CIN], BF16, tag=f"wse2_{k}",
                             name=f"wse2_{k}") for k in range(8)]
        wp = [ipool.tile([128, DIM], BF16, tag=f"wp{p}", name=f"wp{p}")
              for p in range(2)]

        # ---- input DMAs split across the three DGE dispatch engines
        # (SP, ACT, Pool), ordered by consumption: first-exp gates on
        # w_sa1+zT and w_sa2+yT; xh units need wse1+xT; z2 needs wse2.
        for k in range(8):
            nc.sync.dma_start(wsa1_t[k][:], wsa1_d[k * 128:(k + 1) * 128, :])
            nc.scalar.dma_start(wsa2_t[k][:], wsa2_d[k * 128:(k + 1) * 128, :])
        for k in range(4):
            nc.sync.dma_start(yt[k][:], yT_d[k * 128:(k + 1) * 128, :])
            nc.scalar.dma_start(yt[k + 4][:], yT_d[(k + 4) * 128:(k + 5) * 128, :])
        for k in range(8):
            nc.gpsimd.dma_start(zt[k][:], zT_d[k * 128:(k + 1) * 128, :])
        for k in range(8):
            nc.gpsimd.dma_start(wse1_t[k][:], wse1_d[k * 128:(k + 1) * 128, :])
        for k in range(4):
            nc.sync.dma_start(xt[k][:], xT_d[k * 128:(k + 1) * 128, :])
            nc.scalar.dma_start(xt[k + 4][:], xT_d[(k + 4) * 128:(k + 5) * 128, :])
        for k in range(8):
            nc.gpsimd.dma_start(wse2_t[k][:], wse2_d[k * 128:(k + 1) * 128, :])
        for p in range(2):
            nc.scalar.dma_start(wp[p][:], wout_d[p * 128:(p + 1) * 128, :])

        # catp starts at 0 (out1/out2 both accumulate); xh_aug ones columns
        for m in range(2):
            nc.gpsimd.memset(catp[m][:], 0.0)
        for i in range(NCH):
            nc.gpsimd.memset(
                xq[i][:].rearrange("p (h c) -> p h c", c=XP)[:, :, DH:DH + 1],
                1.0)

        # ================= aux PE unit emitters =================
        def emit_z1T(m, nb):
            # z1T pair m, 512-token block nb; psum->SBUF copy on ScalarE
            # (its startup idle window)
            ps = psaux.tile([128, 512], F32, tag="aux", name=f"z1p{m}{nb}")
            mm = None
            for k in range(8):
                mm = nc.tensor.matmul(
                    ps[:],
                    lhsT=wsa1_t[k][:, 128 * m:128 * m + 128],
                    rhs=zt[k][:, 512 * nb:512 * nb + 512],
                    start=(k == 0), stop=(k == 7),
                )
            nc.scalar.copy(z1T[m][:, 512 * nb:512 * nb + 512], ps[:])
            return mm

        def emit_yhT(m, nb):
            ps = psaux.tile([128, 512], F32, tag="aux", name=f"yhp{m}{nb}")
            mm = None
            for k in range(8):
                mm = nc.tensor.matmul(
                    ps[:],
                    lhsT=wsa2_t[k][:, 128 * m:128 * m + 128],
                    rhs=yt[k][:, 512 * nb:512 * nb + 512],
                    start=(k == 0), stop=(k == 7),
                )
            nc.scalar.copy(yhT[m][:, 512 * nb:512 * nb + 512], ps[:])
            return mm

        def emit_xh(i):
            # xh token chunk i -> xh_aug[i] (natural layout, M=128)
            ps = psaux.tile([128, 512], F32, tag="aux", name=f"xhp{i}")
            mm = None
            for k in range(8):
                mm = nc.tensor.matmul(
                    ps[:, 0:CIN],
                    lhsT=xt[k][:, 128 * i:128 * i + 128],
                    rhs=wse1_t[k][:],
                    start=(k == 0), stop=(k == 7),
                )
            src = ps[:, 0:CIN].rearrange("p (h c) -> p h c", c=DH)
            dst = xq[i][:].rearrange("p (h c) -> p h c", c=XP)[:, :, 0:DH]
            nc.vector.tensor_copy(dst, src)
            return mm

        def emit_z2(i):
            ps = psaux.tile([128, 512], F32, tag="aux", name=f"z2p{i}")
            mm = None
            for k in range(8):
                mm = nc.tensor.matmul(
                    ps[:, 0:CIN],
                    lhsT=zt[k][:, 128 * i:128 * i + 128],
                    rhs=wse2_t[k][:],
                    start=(k == 0), stop=(k == 7),
                )
            nc.vector.tensor_copy(zq[i][:, 0:CIN], ps[:, 0:CIN])
            return mm

        def emit_channel():
            # channel-attn logits: the 4 heads' [64,64] accumulation groups
            # ride ONE psum group (rows 0-63, col block 64h per head).
            cmp_ = psaux.tile([128, 512], F32, tag="aux", name="cmps")
            start_mm = None
            chain_last = {}
            mm = None
            for i in range(NCH):
                for h in range(HG):
                    mm = nc.tensor.matmul(
                        cmp_[0:64, 64 * h:64 * h + 64],
                        lhsT=xq[i][:, XP * h:XP * h + DH],
                        rhs=zq[i][:, DH * h:DH * h + DH],
                        start=(i == 0 and h == 0),
                        stop=(i == NCH - 1 and h == HG - 1),
                        skip_group_check=True,
                    )
                    if i == 0 and h == 0:
                        start_mm = mm
                    elif i == 0:
                        _ride(mm, start_mm, "rider after group start")
                    if i == NCH - 1 and h < HG - 1:
                        chain_last[h] = mm
            for h in range(HG - 1):
                _ride(mm, chain_last[h], "stop after rider chains")
            for h in range(HG):
                p_, off = h // 2, 64 * (h % 2)
                st = tpool.tile([64, DH], BF16, tag="cmstage",
                                name=f"cmstage{h}")
                nc.scalar.activation(st[:], cmp_[0:64, 64 * h:64 * h + 64],
                                     EXP, scale=CM_SCALE,
                                     accum_out=rs[h][0:64, 0:1])
                nc.vector.reciprocal(rcm[h][0:64, 0:1], rs[h][0:64, 0:1])
                nc.vector.tensor_scalar_mul(st[:], st[:], rcm[h][0:64, 0:1])
                nc.sync.dma_start(secm_sb[p_][off:off + 64, :], st[:])
            return mm

        def emit_out2(h, nb):
            p_, off = h // 2, 64 * (h % 2)
            pso = psaux.tile([128, 512], F32, tag="aux", name=f"pso{h}{nb}")
            mm = nc.tensor.matmul(
                pso[off:off + 64, :],
                lhsT=secm_sb[p_][off:off + 64, :],
                rhs=yhT[p_][off:off + 64, nb * 512:(nb + 1) * 512],
                start=True, stop=True,
            )
            dst = catp[p_][off:off + 64, nb * 512:(nb + 1) * 512]
            nc.vector.tensor_add(dst, pso[off:off + 64, :], dst)
            return mm

        final_psf = {}

        def emit_final(d, nb, q):
            if q == 0:
                final_psf[(d, nb)] = psaux.tile(
                    [128, 512], F32, tag="aux", name=f"psf{d}{nb}")
            psf = final_psf[(d, nb)]
            mm = nc.tensor.matmul(
                psf[:],
                lhsT=wp[q][:, d * 128:(d + 1) * 128],
                rhs=catp[q][:, nb * 512:(nb + 1) * 512],
                start=(q == 0), stop=(q == 1),
            )
            if q == 1:
                ob = opool.tile([128, 512], BF16, tag="ob", name=f"ob{d}{nb}")
                nc.scalar.copy(ob[:], psf[:])
                nc.sync.dma_start(
                    outT_d[d * 128:(d + 1) * 128, nb * 512:(nb + 1) * 512],
                    ob[:],
                )
            return mm

        # ---- labeled aux queue ----
        # Emission order IS a correctness constraint: Tile only sees writes
        # that were already emitted, so consumers force their producers out
        # of the queue with drain_until() before touching the data.
        aux_thunks = []
        aux_done = set()
        cur_anchor = [None]

        def queue(label, fn, *args):
            aux_thunks.append((label, lambda fn=fn, args=args: fn(*args)))

        def pop_one():
            label, thunk = aux_thunks.pop(0)
            mm = thunk()
            aux_done.add(label)
            if cur_anchor[0] is not None and mm is not None:
                add_dep_helper(mm.ins, cur_anchor[0].ins, sync=False,
                               reason="pin aux to drain slot")

        def drain_aux(k):
            for _ in range(k):
                if aux_thunks:
                    pop_one()

        def drain_until(label):
            while label not in aux_done and aux_thunks:
                pop_one()

        # prologue: just enough for S(p_=0, ib=0) to start
        emit_z1T(0, 0)
        aux_done.add(("z1T", 0, 0))
        emit_yhT(0, 0)
        aux_done.add(("yhT", 0, 0))

        queue(("yhT", 0, 1), emit_yhT, 0, 1)
        queue(("yhT", 0, 2), emit_yhT, 0, 2)
        queue(("yhT", 0, 3), emit_yhT, 0, 3)
        queue(("z1T", 0, 1), emit_z1T, 0, 1)
        queue(("z1T", 0, 2), emit_z1T, 0, 2)
        queue(("z1T", 0, 3), emit_z1T, 0, 3)
        for nb in range(4):
            queue(("yhT", 1, nb), emit_yhT, 1, nb)
        queue(("z1T", 1, 0), emit_z1T, 1, 0)
        queue(("z1T", 1, 1), emit_z1T, 1, 1)
        for i in range(NCH):
            queue(("xh", i), emit_xh, i)
        for i in range(NCH):
            queue(("z2", i), emit_z2, i)
        queue(("ch",), emit_channel)
        for nb in range(4):
            for h in range(HG):
                queue(("out2", h, nb), emit_out2, h, nb)
        queue(("z1T", 1, 2), emit_z1T, 1, 2)
        queue(("z1T", 1, 3), emit_z1T, 1, 3)

        def queue_finals(nb):
            drain_until(("out2", HG - 1, nb))
            for d in range(8):
                for q in range(2):
                    queue(("fin", d, nb, q), emit_final, d, nb, q)

        # ================= spatial attention =================
        pt = {}
        pending = None   # tails of the previous iteration

        def make_tails(p_, ib, avs):
            icol = ib * 512

            def emit():
                # both avsb copies first: they release the avs psum banks
                # that gate the next iteration's AV accumulation groups
                avsbs = []
                for hh in range(2):
                    avsb = tpool.tile([65, 512], F32, tag=f"avsb{hh}",
                                      name=f"avsb{p_}{ib}{hh}")
                    nc.vector.tensor_copy(avsb[:], avs[hh][0:65, :])
                    avsbs.append(avsb)
                for hh in range(2):
                    avsb = avsbs[hh]
                    rc = tpool.tile([1, 512], F32, tag=f"rc{hh}",
                                    name=f"rc{p_}{ib}{hh}")
                    nc.vector.reciprocal(rc[:], avsb[64:65, :])
                    bc = tpool.tile([64, 512], F32, tag=f"bc{hh}",
                                    name=f"bc{p_}{ib}{hh}")
                    nc.gpsimd.partition_broadcast(bc[:], rc[:])
                    if hh == 0:
                        tmp = tpool.tile([64, 512], F32, tag="tmp0",
                                         name=f"tmp{p_}{ib}0")
                        nc.vector.tensor_mul(tmp[:], avsb[0:64, :], bc[:])
                        dst = catp[p_][0:64, icol:icol + 512]
                        nc.vector.tensor_add(dst, tmp[:], dst)
                    else:
                        # catp partitions 64-127: bridge via SBUF->SBUF DMA
                        tmpb = tpool.tile([64, 512], BF16, tag="tmpb",
                                          name=f"tmpb{p_}{ib}")
                        nc.vector.tensor_mul(tmpb[:], avsb[0:64, :], bc[:])
                        hstage = tpool.tile([128, 512], BF16, tag="hstg",
                                            name=f"hstg{p_}{ib}")
                        nc.sync.dma_start(hstage[64:128, :], tmpb[:])
                        dst = catp[p_][64:128, icol:icol + 512]
                        nc.vector.tensor_add(dst, hstage[64:128, :], dst)
            return emit

        first_iter = True
        for p_ in range(2):
            for ib in range(4):
                icol = ib * 512
                drain_until(("z1T", p_, ib))
                avs = [psAV.tile([128, 512], F32, tag="av",
                                 name=f"av{p_}{ib}{hh}") for hh in range(2)]
                av_next = [0]

                def try_av(limit, p_=p_, avs=avs, av_next=av_next):
                    while av_next[0] < limit and \
                            ("xh", av_next[0]) in aux_done:
                        j = av_next[0]
                        for hh in range(2):
                            h = 2 * p_ + hh
                            nc.tensor.matmul(
                                avs[hh][0:DH + 1, :],
                                lhsT=xq[j][:, XP * h:XP * h + DH + 1],
                                rhs=pt[j][:, 512 * hh:512 * hh + 512],
                                start=(j == 0), stop=(j == NCH - 1),
                            )
                        av_next[0] += 1

                for j in range(NCH):
                    if j % 4 == 0:
                        drain_until(("yhT", p_, j // 4))
                    spt = psS.tile([128, 1024], F32, tag="S",
                                   name=f"S{p_}{ib}{j}")
                    s_anchor = None
                    for hh in range(2):
                        off = 64 * hh
                        s_anchor = nc.tensor.matmul(
                            spt[:, 512 * hh:512 * hh + 512],
                            lhsT=yhT[p_][off:off + 64,
                                         j * 128:(j + 1) * 128],
                            rhs=z1T[p_][off:off + 64, icol:icol + 512],
                            start=True, stop=True,
                        )
                    cur_anchor[0] = s_anchor
                    pt[j] = ptpool.tile([128, 1024], BF16, tag="pt",
                                        name=f"pt{p_}{ib}{j}")
                    nc.scalar.activation(pt[j][:], spt[:], EXP, scale=SCALE)
                    if j == 0 and pending is not None:
                        pending()
                        pending = None
                    if j == 8 and p_ == 1 and ib >= 1:
                        queue_finals(ib - 1)
                    try_av(j - 3)
                    drain_aux(2 if (len(aux_thunks) > 48 or
                                    (p_ == 1 and ib == 3)) else 1)
                drain_until(("xh", NCH - 1))
                try_av(NCH)
                pending = make_tails(p_, ib, avs)
                first_iter = False
        cur_anchor[0] = None
        pending()
        queue_finals(3)
        drain_aux(len(aux_thunks))

    nc.compile()
    return nc


_NC_CACHE = {}


def _get_program():
    if "nc" not in _NC_CACHE:
        _NC_CACHE["nc"] = _build_program()
    return _NC_CACHE["nc"]


def _prep_input_maps(x, y, z, w_sa1, w_sa2, w_se1, w_se2, w_out):
    bf16 = lambda a: np.ascontiguousarray(
        np.asarray(a, dtype=np.float32).astype(ml_dtypes.bfloat16))
    fp8 = lambda a: np.ascontiguousarray(
        np.asarray(a, dtype=np.float32).astype(ml_dtypes.float8_e4m3))
    maps = []
    for c in range(NCORES):
        b, g = divmod(c, G)
        sl = slice(g * CIN, (g + 1) * CIN)
        maps.append({
            "xT": fp8(np.asarray(x)[b].T),
            "yT": bf16(np.asarray(y)[b].T),
            "zT": fp8(np.asarray(z)[b].T),
            "w_sa1": bf16(np.asarray(w_sa1)[:, sl]),
            "w_sa2": bf16(np.asarray(w_sa2)[:, sl]),
            "w_se1": bf16(np.asarray(w_se1)[:, sl]),
            "w_se2": bf16(np.asarray(w_se2)[:, sl]),
            "w_out": bf16(np.asarray(w_out)[sl, :]),
        })
    return maps


def run(inputs, trace=False, trace_kwargs=None):
    """Run on hardware; returns (full_output, BassKernelResults)."""
    nc = _get_program()
    in_maps = _prep_input_maps(
        inputs["x"], inputs["y"], inputs["z"],
        inputs["w_sa1"], inputs["w_sa2"], inputs["w_se1"], inputs["w_se2"],
        inputs["w_out"],
    )
    res = run_bass_kernel_spmd(
        nc, in_maps, list(range(NCORES)), trace=trace,
        trace_kwargs=trace_kwargs or {},
    )
    out = np.zeros((B, N, DIM), dtype=np.float32)
    for c in range(NCORES):
        b, _g = divmod(c, G)
        out[b] += np.asarray(res.results[c]["outT"], dtype=np.float32).T
    out += np.asarray(inputs["b_out"], dtype=np.float32)
    return out, res


def kernel(**inputs) -> np.ndarray:
    out, _ = run(inputs, trace=False)
    return out

